# revision 6
# baseline (speedup 1.0000x reference)
"""Trainium2 Bass kernel for nn_AttentionLayer (Luong cross-attention).

reference:
    score[b,e,t] = sum_d enc[b,e,d] * dec[b,t,d]
    P = softmax_e(score)
    ctx[b,t,d]  = sum_e P[b,e,t] * enc[b,e,d]
    out = concat([dec, ctx], axis=-1)

Sharding: data-parallel over batch, one batch element per NeuronCore (8/8).
Host-side prep (sharding/layout only): per-core slices, pre-transposed
[d, e] / [d, t] copies of enc/dec, all cast to bf16 on the host so DMA
lands operands directly in bf16 SBUF tiles with zero on-chip conversion.

bf16 rather than fp32r everywhere: the PE streams 1 row/cycle for both,
but bf16 weight loads get the compiler-automatic Fast Weight Load path
(2 elems/cycle, 4 XBUSes) that fp32r is excluded from, so the LDWEIGHTS
shadow that throttled the fp32r version (~90ns exposed per matmul) drops
to ~half and hides under the PE's 64-deep LDWEIGHTS pull-ahead window.
Accuracy: numpy sim of this exact quantization chain gives rel_fro
6.4e-3 vs the 2e-2 gate.

Per-core algorithm:
  - mm1: S[e_block, t_chunk] = encT.T @ decT -> PSUM  (K = d, two 128-blocks)
  - softmax with a *global shift* instead of a per-column max:
    exp(S - SHIFT) is computed by ACT directly while evicting PSUM->SBUF
    (bias is a per-partition constant, so no reduction pass and no 16MB
    transpose of P is ever needed; P lands straight in the [e, t] layout
    that matmul 2 consumes, already converted to bf16 by ACT).  SHIFT is
    chosen on the host from a row-sampled estimate of max(S); softmax is
    shift-invariant so correctness only needs exp() to stay inside
    bf16/fp32 range, which holds with wide margin.
  - mm2: C[t_block, :] += P_chunk.T @ [enc | 1 | 0]; column 256 accumulates
    Z[t] = sum_e P[e,t] (ones column; the zero pad keeps the row count
    even so the 2-byte rows stay 4-byte aligned).  Final normalize:
    ctx = C[:, :256] * (1/Z), written bf16 and widened on the host.
"""

import numpy as np
import ml_dtypes

B, TE, TD, D = 8, 2048, 2048, 256
P = 128
NE = TE // P          # 16 encoder-time blocks
QW = 512              # decoder-time columns processed per pass
NQ = TD // QW         # 4 passes
TBQ = QW // P         # 4 t-blocks per pass
G = 4                 # e/t-blocks per input DMA chunk

_STATE = {}


def _build_nc(stages=("mm1", "exp", "mm2", "out")):
    import concourse.tile as tile
    from concourse import bacc, mybir

    f32 = mybir.dt.float32
    bf16 = mybir.dt.bfloat16
    EXP = mybir.ActivationFunctionType.Exp

    nc = bacc.Bacc(
        "TRN2",
        target_bir_lowering=False,
        debug=False,
        enable_asserts=False,
    )
    # all three data inputs are pre-cast to bf16 on the host
    enca_d = nc.dram_tensor("enca", [TE, D + 2], bf16, kind="ExternalInput").ap()
    encT_d = nc.dram_tensor("encT", [D, TE], bf16, kind="ExternalInput").ap()
    decT_d = nc.dram_tensor("decT", [D, TD], bf16, kind="ExternalInput").ap()
    shift_d = nc.dram_tensor("shift", [1, 1], f32, kind="ExternalInput").ap()
    ctx_d = nc.dram_tensor("ctx", [TD, D], bf16, kind="ExternalOutput").ap()

    enca_r = enca_d.rearrange("(n p) c -> p n c", p=P)
    encT_r = encT_d.rearrange("(h p) x -> p h x", p=P)
    decT_r = decT_d.rearrange("(h p) x -> p h x", p=P)

    with tile.TileContext(nc) as tc:
        with (
            tc.tile_pool(name="consts", bufs=1) as consts,
            tc.tile_pool(name="pp", bufs=4) as pp,
            tc.tile_pool(name="outp", bufs=4) as outp,
            tc.tile_pool(name="zp", bufs=4) as zp,
            tc.tile_pool(name="ps_s", bufs=3, space="PSUM") as ps_s,
            tc.tile_pool(name="ps_c", bufs=5, space="PSUM") as ps_c,
        ):
            # PE pre-roll: a few throwaway fp32 matmuls with no DMA
            # dependencies.  They pull the PE sequencer's IRAM fetch and
            # sem-wake into the DMA window and start opening the HAM clock
            # gate, so the first real matmul issues ~3us earlier and warmer.
            # They borrow a c-pool PSUM slot, which mm2 only needs later.
            warm = consts.tile([P, P], f32)
            nc.gpsimd.memset(warm[:], 0.0)
            warm_ps = ps_c.tile([P, P], f32, tag="c", name="warm_ps")
            for _ in range(6):
                nc.tensor.matmul(warm_ps[:], warm[:], warm[:], start=True, stop=True)
            # ACT table-load primer: the first ACTIVATE triggers a ~2.7us
            # exp-table DMA; a throwaway exp here runs it during the input
            # DMA window instead of on the exp-chain critical path.
            warm_e = consts.tile([P, 1], f32)
            nc.scalar.activation(warm_e[:], warm[:, 0:1], EXP, bias=0.0, scale=1.0)

            CW = G * P  # 512 columns per chunk
            NC = NE // G  # 4 chunks per tensor

            # One tile per DMA chunk, so a consumer's dependency is exactly
            # its own chunk's transfer (a slice-write into one big tile would
            # leave the first matmul waiting on the whole tensor).
            # chunk 0 is split in half again so mm2 can start on the first
            # two e-blocks while the rest of the head DMAs are in flight
            enc_aug = [
                consts.tile([P, 2, D + 2], bf16, name="enca_c0a"),
                consts.tile([P, 2, D + 2], bf16, name="enca_c0b"),
            ] + [
                consts.tile([P, G, D + 2], bf16, name=f"enca_c{g}")
                for g in range(1, NC)
            ]  # [e%128, e_block%sub, d|1|0]

            def enca_slot(i):
                # map e-block index -> (tile, row) across the uneven split
                if i < 2:
                    return enc_aug[0], i
                if i < 4:
                    return enc_aug[1], i - 2
                return enc_aug[1 + i // G], i % G
            # split further by d-half: the first matmul needs only the h=0
            # halves of decT0/encT0, so halving the critical DMA payload
            encT = [
                [consts.tile([P, CW], bf16, name=f"encT_c{g}h{h}") for h in range(2)]
                for g in range(NC)
            ]  # [g][h]: [d%128, e%CW]
            decT = [
                [consts.tile([P, CW], bf16, name=f"decT_c{g}h{h}") for h in range(2)]
                for g in range(NC)
            ]  # [q][h]: [d%128, t%CW]

            # All input DMAs on the SP HWDGE queue (SP is otherwise idle,
            # and any descriptor generation on ACT would delay the exp
            # chain, which paces mm2).  Quarter 0 of mm1 needs decT cols
            # 0:512 and encT chunks in order, so those go first.
            def dma_T(tiles, src_r, g, h):
                nc.sync.dma_start(
                    out=tiles[g][h][:],
                    in_=src_r[:, h, g * CW : (g + 1) * CW],
                )

            # negative shift first: tiny payload, and its 128 scatter
            # descriptors stay out of the critical chunk stream
            nshift = consts.tile([P, 1], f32)
            nc.sync.dma_start(
                out=nshift[:],
                in_=shift_d.to_broadcast([P, 1]),
            )

            dma_T(decT, decT_r, 0, 0)
            dma_T(encT, encT_r, 0, 0)
            dma_T(decT, decT_r, 0, 1)
            dma_T(encT, encT_r, 0, 1)

            # mm1 consumes encT chunks every ~3.5us -- issue encT1 right
            # behind encT0, ahead of everything mm2/later quarters need.
            dma_T(encT, encT_r, 1, 0)
            dma_T(encT, encT_r, 1, 1)
            nc.sync.dma_start(out=enc_aug[0][:], in_=enca_r[:, 0:2, :])
            nc.sync.dma_start(out=enc_aug[1][:], in_=enca_r[:, 2:G, :])
            dma_T(encT, encT_r, 2, 0)
            dma_T(encT, encT_r, 2, 1)
            nc.sync.dma_start(out=enc_aug[2][:], in_=enca_r[:, G : 2 * G, :])
            dma_T(encT, encT_r, 3, 0)
            dma_T(encT, encT_r, 3, 1)
            for g in range(2, NC):
                nc.sync.dma_start(
                    out=enc_aug[1 + g][:],
                    in_=enca_r[:, g * G : (g + 1) * G, :],
                )
            for g in range(1, NC):
                dma_T(decT, decT_r, g, 0)
                dma_T(decT, decT_r, g, 1)

            def emit_mm1_exp(q, i):
                s = ps_s.tile([P, QW], f32, tag="s", name=f"s{q}_{i}")
                for h in range(2):
                    nc.tensor.matmul(
                        s[:],
                        encT[i // G][h][:, (i % G) * P : (i % G + 1) * P],
                        decT[q][h][:],
                        start=(h == 0),
                        stop=(h == 1),
                    )
                p_t = pp.tile([P, QW], bf16, tag="p", name=f"p{q}_{i}")
                nc.scalar.activation(p_t[:], s[:], EXP, bias=nshift[:], scale=1.0)
                return p_t

            def emit_mm2(q, i, p_t, c_tiles):
                ea, row = enca_slot(i)
                for j in range(TBQ):
                    nc.tensor.matmul(
                        c_tiles[j][:],
                        p_t[:, j * P : (j + 1) * P],
                        ea[:, row, :],
                        start=(i == 0),
                        stop=(i == NE - 1),
                        skip_group_check=True,
                    )

            for q in range(NQ):
                if "mm1" not in stages:
                    continue
                c_tiles = [
                    ps_c.tile([P, D + 2], f32, tag="c", name=f"c{q}_{j}")
                    for j in range(TBQ)
                ]
                # Software pipelining: emit mm1 of iteration i+1 BEFORE mm2 of
                # iteration i.  The PE's queue is in-order, and mm2_i's weight
                # load waits on exp_i (ACT); with the naive order the PE idles
                # for the tail of every exp.  Running the next mm1 (which only
                # needs its own S tile) under exp_i keeps the PE at the row
                # floor; ps_c bufs=8 lets q+1's accumulators start while q's
                # are still being normalized.
                p_prev = None
                for i in range(NE):
                    p_t = emit_mm1_exp(q, i)
                    if "mm2" in stages and p_prev is not None:
                        emit_mm2(q, i - 1, p_prev, c_tiles)
                    p_prev = p_t
                if "mm2" in stages:
                    emit_mm2(q, NE - 1, p_prev, c_tiles)
                if "mm2" not in stages or "out" not in stages:
                    continue
                # one [128, TBQ, D] output tile per quarter -> a single DMA
                # issue instead of four (each DMA_DIRECT2D costs ~600ns of
                # serialized SP issue time, and fewer sems shrink the epilogue)
                o = outp.tile([P, TBQ, D], bf16, tag="o", name=f"o{q}")
                for j in range(TBQ):
                    z = zp.tile([P, 1], f32, tag="z", name=f"z{q}_{j}")
                    nc.vector.reciprocal(z[:], c_tiles[j][:, D : D + 1])
                    if q == NQ - 1 and j % 2 == 1:
                        # ACT is free of exp work by the last quarter; let it
                        # take half the final evictions off DVE's tail.
                        nc.scalar.mul(o[:, j, :], c_tiles[j][:, 0:D], z[:])
                    else:
                        nc.vector.tensor_scalar_mul(o[:, j, :], c_tiles[j][:, 0:D], z[:])
                ctx_q = ctx_d[q * QW : (q + 1) * QW, :].rearrange(
                    "(j p) d -> p j d", p=P
                )
                nc.sync.dma_start(out=ctx_q, in_=o[:])

    nc.compile()
    return nc


def _get_nc():
    if "nc" not in _STATE:
        _STATE["nc"] = _build_nc()
    return _STATE["nc"]


def _bf16(x):
    """Fast round-to-nearest-even fp32 -> bf16 via integer ops."""
    u = np.ascontiguousarray(x, dtype=np.float32).view(np.uint32)
    r = ((u + np.uint32(0x7FFF) + ((u >> np.uint32(16)) & np.uint32(1)))
         >> np.uint32(16)).astype(np.uint16)
    return r.view(ml_dtypes.bfloat16)


def _pick_shift(enc, dec):
    """Row-sampled estimate of max(score) + margin.  Softmax is invariant to
    the shift; it only has to keep every exp() finite (shift >= max-88) while
    not flushing the per-column dominant terms to zero (shift <= colmax+80).
    A sampled global max + 4 sits inside that window with wide margin for
    any gaussian-ish score distribution (sampling underestimates the true
    max by far less than the fp32/bf16-exp headroom the bound needs)."""
    rng = np.random.default_rng(0)
    rows = rng.choice(TE, size=32, replace=False)
    samp = np.einsum("bed,btd->bet", enc[:, rows, :], dec, optimize=True)
    return float(samp.max()) + 4.0


def _in_maps(enc, dec):
    nshift = np.full((1, 1), -_pick_shift(enc, dec), dtype=np.float32)
    maps = []
    for b in range(B):
        enca = np.zeros((TE, D + 2), dtype=np.float32)
        enca[:, :D] = enc[b]
        enca[:, D] = 1.0
        maps.append(
            {
                "enca": _bf16(enca),
                "encT": _bf16(np.ascontiguousarray(enc[b].T)),
                "decT": _bf16(np.ascontiguousarray(dec[b].T)),
                "shift": nshift,
            }
        )
    return maps


def kernel(encoder_outputs, decoder_outputs):
    from concourse.bass_utils import run_bass_kernel_spmd

    enc = np.ascontiguousarray(np.asarray(encoder_outputs, dtype=np.float32))
    dec = np.ascontiguousarray(np.asarray(decoder_outputs, dtype=np.float32))
    assert enc.shape == (B, TE, D) and dec.shape == (B, TD, D)

    nc = _get_nc()
    res = run_bass_kernel_spmd(nc, _in_maps(enc, dec), list(range(B))).results
    ctx = np.stack(
        [np.asarray(res[b]["ctx"]).astype(np.float32) for b in range(B)], axis=0
    )
    return np.concatenate([dec, ctx], axis=-1)


# revision 7
# speedup vs baseline: 1.1701x; 1.1701x over previous
"""Trainium2 Bass kernel for nn_AttentionLayer (Luong cross-attention).

reference:
    score[b,e,t] = sum_d enc[b,e,d] * dec[b,t,d]
    P = softmax_e(score)
    ctx[b,t,d]  = sum_e P[b,e,t] * enc[b,e,d]
    out = concat([dec, ctx], axis=-1)

Sharding: data-parallel over batch, one batch element per NeuronCore (8/8).
Host-side prep (sharding/layout only): per-core slices, pre-transposed
[d, e] / [d, t] copies of enc/dec, all cast to bf16 on the host so DMA
lands operands directly in bf16 SBUF tiles with zero on-chip conversion.

bf16 rather than fp32r everywhere: the PE streams 1 row/cycle for both,
but bf16 weight loads get the compiler-automatic Fast Weight Load path
(2 elems/cycle, 4 XBUSes) that fp32r is excluded from, so the LDWEIGHTS
shadow that throttled the fp32r version (~90ns exposed per matmul) drops
to ~half and hides under the PE's 64-deep LDWEIGHTS pull-ahead window.
Accuracy: numpy sim of this exact quantization chain gives rel_fro
6.4e-3 vs the 2e-2 gate.

Per-core algorithm:
  - mm1: S[e_block, t_chunk] = encT.T @ decT -> PSUM  (K = d, two 128-blocks)
  - softmax with a *global shift* instead of a per-column max:
    exp(S - SHIFT) is computed by ACT directly while evicting PSUM->SBUF
    (bias is a per-partition constant, so no reduction pass and no 16MB
    transpose of P is ever needed; P lands straight in the [e, t] layout
    that matmul 2 consumes, already converted to bf16 by ACT).  SHIFT is
    chosen on the host from a row-sampled estimate of max(S); softmax is
    shift-invariant so correctness only needs exp() to stay inside
    bf16/fp32 range, which holds with wide margin.
  - mm2: C[t_block, :] += P_chunk.T @ [enc | 1 | 0]; column 256 accumulates
    Z[t] = sum_e P[e,t] (ones column; the zero pad keeps the row count
    even so the 2-byte rows stay 4-byte aligned).  Final normalize:
    ctx = C[:, :256] * (1/Z), written bf16 and widened on the host.
"""

import numpy as np
import ml_dtypes

B, TE, TD, D = 8, 2048, 2048, 256
P = 128
NE = TE // P          # 16 encoder-time blocks
QW = 512              # decoder-time columns processed per pass
NQ = TD // QW         # 4 passes
TBQ = QW // P         # 4 t-blocks per pass
G = 4                 # e/t-blocks per input DMA chunk

_STATE = {}


def _build_nc(stages=("mm1", "exp", "mm2", "out")):
    import concourse.tile as tile
    from concourse import bacc, mybir

    f32 = mybir.dt.float32
    bf16 = mybir.dt.bfloat16
    EXP = mybir.ActivationFunctionType.Exp

    nc = bacc.Bacc(
        "TRN2",
        target_bir_lowering=False,
        debug=False,
        enable_asserts=False,
    )
    # all three data inputs are pre-cast to bf16 on the host
    enca_d = nc.dram_tensor("enca", [TE, D + 2], bf16, kind="ExternalInput").ap()
    encT_d = nc.dram_tensor("encT", [D, TE], bf16, kind="ExternalInput").ap()
    decT_d = nc.dram_tensor("decT", [D, TD], bf16, kind="ExternalInput").ap()
    shift_d = nc.dram_tensor("shift", [1, 1], f32, kind="ExternalInput").ap()
    ctx_d = nc.dram_tensor("ctx", [TD, D], bf16, kind="ExternalOutput").ap()

    enca_r = enca_d.rearrange("(n p) c -> p n c", p=P)
    encT_r = encT_d.rearrange("(h p) x -> p h x", p=P)
    decT_r = decT_d.rearrange("(h p) x -> p h x", p=P)

    with tile.TileContext(nc) as tc:
        with (
            tc.tile_pool(name="consts", bufs=1) as consts,
            tc.tile_pool(name="pp", bufs=4) as pp,
            tc.tile_pool(name="outp", bufs=4) as outp,
            tc.tile_pool(name="zp", bufs=4) as zp,
            tc.tile_pool(name="ps_s", bufs=3, space="PSUM") as ps_s,
            tc.tile_pool(name="ps_c", bufs=5, space="PSUM") as ps_c,
        ):
            # PE pre-roll: a few throwaway fp32 matmuls with no DMA
            # dependencies.  They pull the PE sequencer's IRAM fetch and
            # sem-wake into the DMA window and start opening the HAM clock
            # gate, so the first real matmul issues ~3us earlier and warmer.
            # They borrow a c-pool PSUM slot, which mm2 only needs later.
            warm = consts.tile([P, P], f32)
            nc.gpsimd.memset(warm[:], 0.0)
            warm_ps = ps_c.tile([P, P], f32, tag="c", name="warm_ps")
            for _ in range(6):
                nc.tensor.matmul(warm_ps[:], warm[:], warm[:], start=True, stop=True)
            # ACT table-load primer: the first ACTIVATE triggers a ~2.7us
            # exp-table DMA; a throwaway exp here runs it during the input
            # DMA window instead of on the exp-chain critical path.
            warm_e = consts.tile([P, 1], f32)
            nc.scalar.activation(warm_e[:], warm[:, 0:1], EXP, bias=0.0, scale=1.0)

            CW = G * P  # 512 columns per chunk
            NC = NE // G  # 4 chunks per tensor

            # One tile per DMA chunk, so a consumer's dependency is exactly
            # its own chunk's transfer (a slice-write into one big tile would
            # leave the first matmul waiting on the whole tensor).
            # chunk 0 is split in half again so mm2 can start on the first
            # two e-blocks while the rest of the head DMAs are in flight
            enc_aug = [
                consts.tile([P, 2, D + 2], bf16, name="enca_c0a"),
                consts.tile([P, 2, D + 2], bf16, name="enca_c0b"),
            ] + [
                consts.tile([P, G, D + 2], bf16, name=f"enca_c{g}")
                for g in range(1, NC)
            ]  # [e%128, e_block%sub, d|1|0]

            def enca_slot(i):
                # map e-block index -> (tile, row) across the uneven split
                if i < 2:
                    return enc_aug[0], i
                if i < 4:
                    return enc_aug[1], i - 2
                return enc_aug[1 + i // G], i % G
            # split further by d-half: the first matmul needs only the h=0
            # halves of decT0/encT0, so halving the critical DMA payload
            encT = [
                [consts.tile([P, CW], bf16, name=f"encT_c{g}h{h}") for h in range(2)]
                for g in range(NC)
            ]  # [g][h]: [d%128, e%CW]
            decT = [
                [consts.tile([P, CW], bf16, name=f"decT_c{g}h{h}") for h in range(2)]
                for g in range(NC)
            ]  # [q][h]: [d%128, t%CW]

            # All input DMAs on the SP HWDGE queue (SP is otherwise idle,
            # and any descriptor generation on ACT would delay the exp
            # chain, which paces mm2).  Quarter 0 of mm1 needs decT cols
            # 0:512 and encT chunks in order, so those go first.
            def dma_T(tiles, src_r, g, h):
                nc.sync.dma_start(
                    out=tiles[g][h][:],
                    in_=src_r[:, h, g * CW : (g + 1) * CW],
                )

            # negative shift first: tiny payload, and its 128 scatter
            # descriptors stay out of the critical chunk stream
            nshift = consts.tile([P, 1], f32)
            nc.sync.dma_start(
                out=nshift[:],
                in_=shift_d.to_broadcast([P, 1]),
            )

            dma_T(decT, decT_r, 0, 0)
            dma_T(encT, encT_r, 0, 0)
            dma_T(decT, decT_r, 0, 1)
            dma_T(encT, encT_r, 0, 1)

            # enca chunk 0a feeds mm2 of the very first pipeline iteration
            # (~1 period after mm1 starts); issuing it 7th left mm2 stalled
            # ~1.8us on its transfer, so it goes right behind the mm1 head.
            nc.sync.dma_start(out=enc_aug[0][:], in_=enca_r[:, 0:2, :])
            # mm1 consumes encT chunks every ~3.5us -- issue encT1 right
            # behind encT0, ahead of everything mm2/later quarters need.
            dma_T(encT, encT_r, 1, 0)
            dma_T(encT, encT_r, 1, 1)
            nc.sync.dma_start(out=enc_aug[1][:], in_=enca_r[:, 2:G, :])
            dma_T(encT, encT_r, 2, 0)
            dma_T(encT, encT_r, 2, 1)
            nc.sync.dma_start(out=enc_aug[2][:], in_=enca_r[:, G : 2 * G, :])
            dma_T(encT, encT_r, 3, 0)
            dma_T(encT, encT_r, 3, 1)
            for g in range(2, NC):
                nc.sync.dma_start(
                    out=enc_aug[1 + g][:],
                    in_=enca_r[:, g * G : (g + 1) * G, :],
                )
            for g in range(1, NC):
                dma_T(decT, decT_r, g, 0)
                dma_T(decT, decT_r, g, 1)

            def emit_mm1_exp(q, i):
                s = ps_s.tile([P, QW], f32, tag="s", name=f"s{q}_{i}")
                for h in range(2):
                    nc.tensor.matmul(
                        s[:],
                        encT[i // G][h][:, (i % G) * P : (i % G + 1) * P],
                        decT[q][h][:],
                        start=(h == 0),
                        stop=(h == 1),
                    )
                p_t = pp.tile([P, QW], bf16, tag="p", name=f"p{q}_{i}")
                nc.scalar.activation(p_t[:], s[:], EXP, bias=nshift[:], scale=1.0)
                return p_t

            def emit_mm2(q, i, p_t, c_tiles):
                ea, row = enca_slot(i)
                for j in range(TBQ):
                    nc.tensor.matmul(
                        c_tiles[j][:],
                        p_t[:, j * P : (j + 1) * P],
                        ea[:, row, :],
                        start=(i == 0),
                        stop=(i == NE - 1),
                        skip_group_check=True,
                    )

            for q in range(NQ):
                if "mm1" not in stages:
                    continue
                c_tiles = [
                    ps_c.tile([P, D + 2], f32, tag="c", name=f"c{q}_{j}")
                    for j in range(TBQ)
                ]
                # Software pipelining: emit mm1 of iteration i+1 BEFORE mm2 of
                # iteration i.  The PE's queue is in-order, and mm2_i's weight
                # load waits on exp_i (ACT); with the naive order the PE idles
                # for the tail of every exp.  Running the next mm1 (which only
                # needs its own S tile) under exp_i keeps the PE at the row
                # floor; ps_c bufs=8 lets q+1's accumulators start while q's
                # are still being normalized.
                p_prev = None
                for i in range(NE):
                    p_t = emit_mm1_exp(q, i)
                    if "mm2" in stages and p_prev is not None:
                        emit_mm2(q, i - 1, p_prev, c_tiles)
                    p_prev = p_t
                if "mm2" in stages:
                    emit_mm2(q, NE - 1, p_prev, c_tiles)
                if "mm2" not in stages or "out" not in stages:
                    continue
                # one [128, TBQ, D] output tile per quarter -> a single DMA
                # issue instead of four (each DMA_DIRECT2D costs ~600ns of
                # serialized SP issue time, and fewer sems shrink the epilogue)
                o = outp.tile([P, TBQ, D], bf16, tag="o", name=f"o{q}")
                for j in range(TBQ):
                    z = zp.tile([P, 1], f32, tag="z", name=f"z{q}_{j}")
                    nc.vector.reciprocal(z[:], c_tiles[j][:, D : D + 1])
                    if q == NQ - 1 and j % 2 == 1:
                        # ACT is free of exp work by the last quarter; let it
                        # take half the final evictions off DVE's tail.
                        nc.scalar.mul(o[:, j, :], c_tiles[j][:, 0:D], z[:])
                    else:
                        nc.vector.tensor_scalar_mul(o[:, j, :], c_tiles[j][:, 0:D], z[:])
                ctx_q = ctx_d[q * QW : (q + 1) * QW, :].rearrange(
                    "(j p) d -> p j d", p=P
                )
                nc.sync.dma_start(out=ctx_q, in_=o[:])

    nc.compile()
    return nc


def _get_nc():
    if "nc" not in _STATE:
        _STATE["nc"] = _build_nc()
    return _STATE["nc"]


def _bf16(x):
    """Fast round-to-nearest-even fp32 -> bf16 via integer ops."""
    u = np.ascontiguousarray(x, dtype=np.float32).view(np.uint32)
    r = ((u + np.uint32(0x7FFF) + ((u >> np.uint32(16)) & np.uint32(1)))
         >> np.uint32(16)).astype(np.uint16)
    return r.view(ml_dtypes.bfloat16)


def _pick_shift(enc, dec):
    """Row-sampled estimate of max(score) + margin.  Softmax is invariant to
    the shift; it only has to keep every exp() finite (shift >= max-88) while
    not flushing the per-column dominant terms to zero (shift <= colmax+80).
    A sampled global max + 4 sits inside that window with wide margin for
    any gaussian-ish score distribution (sampling underestimates the true
    max by far less than the fp32/bf16-exp headroom the bound needs)."""
    rng = np.random.default_rng(0)
    rows = rng.choice(TE, size=32, replace=False)
    samp = np.einsum("bed,btd->bet", enc[:, rows, :], dec, optimize=True)
    return float(samp.max()) + 4.0


def _in_maps(enc, dec):
    nshift = np.full((1, 1), -_pick_shift(enc, dec), dtype=np.float32)
    maps = []
    for b in range(B):
        enca = np.zeros((TE, D + 2), dtype=np.float32)
        enca[:, :D] = enc[b]
        enca[:, D] = 1.0
        maps.append(
            {
                "enca": _bf16(enca),
                "encT": _bf16(np.ascontiguousarray(enc[b].T)),
                "decT": _bf16(np.ascontiguousarray(dec[b].T)),
                "shift": nshift,
            }
        )
    return maps


def kernel(encoder_outputs, decoder_outputs):
    from concourse.bass_utils import run_bass_kernel_spmd

    enc = np.ascontiguousarray(np.asarray(encoder_outputs, dtype=np.float32))
    dec = np.ascontiguousarray(np.asarray(decoder_outputs, dtype=np.float32))
    assert enc.shape == (B, TE, D) and dec.shape == (B, TD, D)

    nc = _get_nc()
    res = run_bass_kernel_spmd(nc, _in_maps(enc, dec), list(range(B))).results
    ctx = np.stack(
        [np.asarray(res[b]["ctx"]).astype(np.float32) for b in range(B)], axis=0
    )
    return np.concatenate([dec, ctx], axis=-1)


# revision 9
# speedup vs baseline: 1.1710x; 1.0007x over previous
"""Trainium2 Bass kernel for nn_AttentionLayer (Luong cross-attention).

reference:
    score[b,e,t] = sum_d enc[b,e,d] * dec[b,t,d]
    P = softmax_e(score)
    ctx[b,t,d]  = sum_e P[b,e,t] * enc[b,e,d]
    out = concat([dec, ctx], axis=-1)

Sharding: data-parallel over batch, one batch element per NeuronCore (8/8).
Host-side prep (sharding/layout only): per-core slices, pre-transposed
[d, e] / [d, t] copies of enc/dec, all cast to bf16 on the host so DMA
lands operands directly in bf16 SBUF tiles with zero on-chip conversion.

bf16 rather than fp32r everywhere: the PE streams 1 row/cycle for both,
but bf16 weight loads get the compiler-automatic Fast Weight Load path
(2 elems/cycle, 4 XBUSes) that fp32r is excluded from, so the LDWEIGHTS
shadow that throttled the fp32r version (~90ns exposed per matmul) drops
to ~half and hides under the PE's 64-deep LDWEIGHTS pull-ahead window.
Accuracy: numpy sim of this exact quantization chain gives rel_fro
6.4e-3 vs the 2e-2 gate.

Per-core algorithm:
  - mm1: S[e_block, t_chunk] = encT.T @ decT -> PSUM  (K = d, two 128-blocks)
  - softmax with a *global shift* instead of a per-column max:
    exp(S - SHIFT) is computed by ACT directly while evicting PSUM->SBUF
    (bias is a per-partition constant, so no reduction pass and no 16MB
    transpose of P is ever needed; P lands straight in the [e, t] layout
    that matmul 2 consumes, already converted to bf16 by ACT).  SHIFT is
    chosen on the host from a row-sampled estimate of max(S); softmax is
    shift-invariant so correctness only needs exp() to stay inside
    bf16/fp32 range, which holds with wide margin.
  - mm2: C[t_block, :] += P_chunk.T @ [enc | 1 | 0]; column 256 accumulates
    Z[t] = sum_e P[e,t] (ones column; the zero pad keeps the row count
    even so the 2-byte rows stay 4-byte aligned).  Final normalize:
    ctx = C[:, :256] * (1/Z), written bf16 and widened on the host.
"""

import numpy as np
import ml_dtypes

B, TE, TD, D = 8, 2048, 2048, 256
P = 128
NE = TE // P          # 16 encoder-time blocks
QW = 512              # decoder-time columns processed per pass
NQ = TD // QW         # 4 passes
TBQ = QW // P         # 4 t-blocks per pass
G = 4                 # e/t-blocks per input DMA chunk

_STATE = {}


def _build_nc(stages=("mm1", "exp", "mm2", "out")):
    import concourse.tile as tile
    from concourse import bacc, mybir

    f32 = mybir.dt.float32
    bf16 = mybir.dt.bfloat16
    EXP = mybir.ActivationFunctionType.Exp

    nc = bacc.Bacc(
        "TRN2",
        target_bir_lowering=False,
        debug=False,
        enable_asserts=False,
    )
    # all three data inputs are pre-cast to bf16 on the host
    enca_d = nc.dram_tensor("enca", [TE, D + 2], bf16, kind="ExternalInput").ap()
    encT_d = nc.dram_tensor("encT", [D, TE], bf16, kind="ExternalInput").ap()
    decT_d = nc.dram_tensor("decT", [D, TD], bf16, kind="ExternalInput").ap()
    shift_d = nc.dram_tensor("shift", [1, 1], f32, kind="ExternalInput").ap()
    ctx_d = nc.dram_tensor("ctx", [TD, D], bf16, kind="ExternalOutput").ap()

    enca_r = enca_d.rearrange("(n p) c -> p n c", p=P)
    encT_r = encT_d.rearrange("(h p) x -> p h x", p=P)
    decT_r = decT_d.rearrange("(h p) x -> p h x", p=P)

    with tile.TileContext(nc) as tc:
        with (
            tc.tile_pool(name="consts", bufs=1) as consts,
            tc.tile_pool(name="pp", bufs=4) as pp,
            tc.tile_pool(name="outp", bufs=4) as outp,
            tc.tile_pool(name="zp", bufs=4) as zp,
            tc.tile_pool(name="ps_s", bufs=3, space="PSUM") as ps_s,
            tc.tile_pool(name="ps_c", bufs=5, space="PSUM") as ps_c,
        ):
            # PE pre-roll: a few throwaway fp32 matmuls with no DMA
            # dependencies.  They pull the PE sequencer's IRAM fetch and
            # sem-wake into the DMA window and start opening the HAM clock
            # gate, so the first real matmul issues ~3us earlier and warmer.
            # They borrow a c-pool PSUM slot, which mm2 only needs later.
            warm = consts.tile([P, P], f32)
            nc.gpsimd.memset(warm[:], 0.0)
            warm_ps = ps_c.tile([P, P], f32, tag="c", name="warm_ps")
            for _ in range(6):
                nc.tensor.matmul(warm_ps[:], warm[:], warm[:], start=True, stop=True)
            # ACT table-load primer: the first ACTIVATE triggers a ~2.7us
            # exp-table DMA; a throwaway exp here runs it during the input
            # DMA window instead of on the exp-chain critical path.
            warm_e = consts.tile([P, 1], f32)
            nc.scalar.activation(warm_e[:], warm[:, 0:1], EXP, bias=0.0, scale=1.0)

            CW = G * P  # 512 columns per chunk
            NC = NE // G  # 4 chunks per tensor

            # One tile per DMA chunk, so a consumer's dependency is exactly
            # its own chunk's transfer (a slice-write into one big tile would
            # leave the first matmul waiting on the whole tensor).
            # chunk 0 is split in half again so mm2 can start on the first
            # two e-blocks while the rest of the head DMAs are in flight
            enc_aug = [
                consts.tile([P, 2, D + 2], bf16, name="enca_c0a"),
                consts.tile([P, 2, D + 2], bf16, name="enca_c0b"),
            ] + [
                consts.tile([P, G, D + 2], bf16, name=f"enca_c{g}")
                for g in range(1, NC)
            ]  # [e%128, e_block%sub, d|1|0]

            def enca_slot(i):
                # map e-block index -> (tile, row) across the uneven split
                if i < 2:
                    return enc_aug[0], i
                if i < 4:
                    return enc_aug[1], i - 2
                return enc_aug[1 + i // G], i % G
            # split further by d-half: the first matmul needs only the h=0
            # halves of decT0/encT0, so halving the critical DMA payload
            encT = [
                [consts.tile([P, CW], bf16, name=f"encT_c{g}h{h}") for h in range(2)]
                for g in range(NC)
            ]  # [g][h]: [d%128, e%CW]
            decT = [
                [consts.tile([P, CW], bf16, name=f"decT_c{g}h{h}") for h in range(2)]
                for g in range(NC)
            ]  # [q][h]: [d%128, t%CW]

            # All input DMAs on the SP HWDGE queue (SP is otherwise idle,
            # and any descriptor generation on ACT would delay the exp
            # chain, which paces mm2).  Quarter 0 of mm1 needs decT cols
            # 0:512 and encT chunks in order, so those go first.
            def dma_T(tiles, src_r, g, h):
                nc.sync.dma_start(
                    out=tiles[g][h][:],
                    in_=src_r[:, h, g * CW : (g + 1) * CW],
                )

            # negative shift first: tiny payload, and its 128 scatter
            # descriptors stay out of the critical chunk stream
            nshift = consts.tile([P, 1], f32)
            nc.sync.dma_start(
                out=nshift[:],
                in_=shift_d.to_broadcast([P, 1]),
            )

            dma_T(decT, decT_r, 0, 0)
            dma_T(encT, encT_r, 0, 0)
            dma_T(decT, decT_r, 0, 1)
            dma_T(encT, encT_r, 0, 1)

            # enca chunk 0a feeds mm2 of the very first pipeline iteration
            # (~1 period after mm1 starts); issuing it 7th left mm2 stalled
            # ~1.8us on its transfer, so it goes right after the mm1 head
            # (but behind the h1 tiles, which the second real matmul needs).
            nc.sync.dma_start(out=enc_aug[0][:], in_=enca_r[:, 0:2, :])
            # mm1 consumes encT chunks every ~3.5us -- issue encT1 right
            # behind encT0, ahead of everything mm2/later quarters need.
            dma_T(encT, encT_r, 1, 0)
            dma_T(encT, encT_r, 1, 1)
            nc.sync.dma_start(out=enc_aug[1][:], in_=enca_r[:, 2:G, :])

            dma_T(encT, encT_r, 2, 0)
            dma_T(encT, encT_r, 2, 1)
            nc.sync.dma_start(out=enc_aug[2][:], in_=enca_r[:, G : 2 * G, :])
            dma_T(encT, encT_r, 3, 0)
            dma_T(encT, encT_r, 3, 1)
            for g in range(2, NC):
                nc.sync.dma_start(
                    out=enc_aug[1 + g][:],
                    in_=enca_r[:, g * G : (g + 1) * G, :],
                )
            for g in range(1, NC):
                dma_T(decT, decT_r, g, 0)
                dma_T(decT, decT_r, g, 1)

            def emit_mm1_exp(q, i):
                s = ps_s.tile([P, QW], f32, tag="s", name=f"s{q}_{i}")
                for h in range(2):
                    nc.tensor.matmul(
                        s[:],
                        encT[i // G][h][:, (i % G) * P : (i % G + 1) * P],
                        decT[q][h][:],
                        start=(h == 0),
                        stop=(h == 1),
                    )
                p_t = pp.tile([P, QW], bf16, tag="p", name=f"p{q}_{i}")
                nc.scalar.activation(p_t[:], s[:], EXP, bias=nshift[:], scale=1.0)
                return p_t

            def emit_mm2(q, i, p_t, c_tiles):
                ea, row = enca_slot(i)
                for j in range(TBQ):
                    nc.tensor.matmul(
                        c_tiles[j][:],
                        p_t[:, j * P : (j + 1) * P],
                        ea[:, row, :],
                        start=(i == 0),
                        stop=(i == NE - 1),
                        skip_group_check=True,
                    )

            for q in range(NQ):
                if "mm1" not in stages:
                    continue
                c_tiles = [
                    ps_c.tile([P, D + 2], f32, tag="c", name=f"c{q}_{j}")
                    for j in range(TBQ)
                ]
                # Software pipelining: emit mm1 of iteration i+1 BEFORE mm2 of
                # iteration i.  The PE's queue is in-order, and mm2_i's weight
                # load waits on exp_i (ACT); with the naive order the PE idles
                # for the tail of every exp.  Running the next mm1 (which only
                # needs its own S tile) under exp_i keeps the PE at the row
                # floor; ps_c bufs=8 lets q+1's accumulators start while q's
                # are still being normalized.
                p_prev = None
                for i in range(NE):
                    p_t = emit_mm1_exp(q, i)
                    if "mm2" in stages and p_prev is not None:
                        emit_mm2(q, i - 1, p_prev, c_tiles)
                    p_prev = p_t
                if "mm2" in stages:
                    emit_mm2(q, NE - 1, p_prev, c_tiles)
                if "mm2" not in stages or "out" not in stages:
                    continue
                if q < NQ - 1:
                    # one [128, TBQ, D] output tile per quarter -> a single
                    # DMA issue instead of four (each DMA_DIRECT2D costs
                    # ~600ns of serialized SP issue time); fully overlapped
                    # by the next quarter's compute.
                    o = outp.tile([P, TBQ, D], bf16, tag="o", name=f"o{q}")
                    for j in range(TBQ):
                        z = zp.tile([P, 1], f32, tag="z", name=f"z{q}_{j}")
                        nc.vector.reciprocal(z[:], c_tiles[j][:, D : D + 1])
                        nc.vector.tensor_scalar_mul(
                            o[:, j, :], c_tiles[j][:, 0:D], z[:]
                        )
                    ctx_q = ctx_d[q * QW : (q + 1) * QW, :].rearrange(
                        "(j p) d -> p j d", p=P
                    )
                    nc.sync.dma_start(out=ctx_q, in_=o[:])
                else:
                    # last quarter is the drain tail: ship each 128-row slab
                    # as soon as its normalize finishes, alternating DVE/ACT
                    # (ACT is free of exp work here) so two slabs progress
                    # in parallel.
                    for j in range(TBQ):
                        z = zp.tile([P, 1], f32, tag="z", name=f"z{q}_{j}")
                        nc.vector.reciprocal(z[:], c_tiles[j][:, D : D + 1])
                        o = outp.tile([P, D], bf16, tag="o", name=f"o{q}_{j}")
                        if j % 2 == 0:
                            nc.scalar.mul(o[:], c_tiles[j][:, 0:D], z[:])
                        else:
                            nc.vector.tensor_scalar_mul(o[:], c_tiles[j][:, 0:D], z[:])
                        t0 = (q * TBQ + j) * P
                        nc.sync.dma_start(out=ctx_d[t0 : t0 + P, :], in_=o[:])

    nc.compile()
    return nc


def _get_nc():
    if "nc" not in _STATE:
        _STATE["nc"] = _build_nc()
    return _STATE["nc"]


def _bf16(x):
    """Fast round-to-nearest-even fp32 -> bf16 via integer ops."""
    u = np.ascontiguousarray(x, dtype=np.float32).view(np.uint32)
    r = ((u + np.uint32(0x7FFF) + ((u >> np.uint32(16)) & np.uint32(1)))
         >> np.uint32(16)).astype(np.uint16)
    return r.view(ml_dtypes.bfloat16)


def _pick_shift(enc, dec):
    """Row-sampled estimate of max(score) + margin.  Softmax is invariant to
    the shift; it only has to keep every exp() finite (shift >= max-88) while
    not flushing the per-column dominant terms to zero (shift <= colmax+80).
    A sampled global max + 4 sits inside that window with wide margin for
    any gaussian-ish score distribution (sampling underestimates the true
    max by far less than the fp32/bf16-exp headroom the bound needs)."""
    rng = np.random.default_rng(0)
    rows = rng.choice(TE, size=32, replace=False)
    samp = np.einsum("bed,btd->bet", enc[:, rows, :], dec, optimize=True)
    return float(samp.max()) + 4.0


def _in_maps(enc, dec):
    nshift = np.full((1, 1), -_pick_shift(enc, dec), dtype=np.float32)
    maps = []
    for b in range(B):
        enca = np.zeros((TE, D + 2), dtype=np.float32)
        enca[:, :D] = enc[b]
        enca[:, D] = 1.0
        maps.append(
            {
                "enca": _bf16(enca),
                "encT": _bf16(np.ascontiguousarray(enc[b].T)),
                "decT": _bf16(np.ascontiguousarray(dec[b].T)),
                "shift": nshift,
            }
        )
    return maps


def kernel(encoder_outputs, decoder_outputs):
    from concourse.bass_utils import run_bass_kernel_spmd

    enc = np.ascontiguousarray(np.asarray(encoder_outputs, dtype=np.float32))
    dec = np.ascontiguousarray(np.asarray(decoder_outputs, dtype=np.float32))
    assert enc.shape == (B, TE, D) and dec.shape == (B, TD, D)

    nc = _get_nc()
    res = run_bass_kernel_spmd(nc, _in_maps(enc, dec), list(range(B))).results
    ctx = np.stack(
        [np.asarray(res[b]["ctx"]).astype(np.float32) for b in range(B)], axis=0
    )
    return np.concatenate([dec, ctx], axis=-1)
